# revision 59
# baseline (speedup 1.0000x reference)
"""CoreAttention Trainium2 Bass kernel.

Full inputs -> full output; internally shards (batch, head-group) across 8
NeuronCores: core c handles batch c//4, heads 4*(c%4) .. 4*(c%4)+4.

Per-core algorithm (per head, seq=2048, d=128):
  - scores are computed TRANSPOSED: S^T[k, q] = (K^T).T @ (Q^T) on the PE,
    so that softmax probabilities come out directly in the [k, q] layout that
    the second matmul (context = P @ V) needs as its stationary operand.
  - Q^T and K^T are prepared HOST-SIDE as fp16 [d, s] per head, so the device
    does no transposes and no dtype converts: DMA loads feed the PE directly.
  - softmax skips max-subtraction (logits ~ N(0,1); exp is safe in fp32) and
    the row sums come for free from a ones-column appended to V.  Masked
    entries are zeroed exactly after exp (matching the reference where
    exp(-10000 - max) underflows to 0), and normalization happens on the
    [q, 128] context output via a per-row reciprocal.
  - the boolean mask is converted host-side to a TRANSPOSED fp16
    keep-multiplier [sk, sq] (1.0 = unmasked) and loaded with natural-layout
    DMA into per-k-tile [k, q] multiplier tiles.
  - PE operands are fp16 (1 cycle/row); accumulation is fp32 in PSUM.
"""

from contextlib import ExitStack

import numpy as np

import concourse.bacc as bacc
from concourse import mybir
import concourse.tile as tile
from concourse.bass_utils import run_bass_kernel_spmd
from concourse.masks import make_identity

S, B, H, D = 2048, 2, 16, 128
HPC = 4  # heads per core
N_CORES = 8
P = 128
NT = S // P  # 16 key/query tiles
SCALE = float(1.0 / np.sqrt(D))  # norm_factor = sqrt(d) * layer_number(=1)

f32 = mybir.dt.float32
f16 = mybir.dt.float16

Exp = mybir.ActivationFunctionType.Exp
MUL = mybir.AluOpType.mult


def _emit(ctx, tc, qt_d, kt_d, v_d, m_d, o_d, reps=1, hw_loop=False, ablate=()):
    nc = tc.nc
    const = ctx.enter_context(tc.tile_pool(name="const", bufs=1))
    predp = ctx.enter_context(tc.tile_pool(name="pred", bufs=1))
    stg = ctx.enter_context(tc.tile_pool(name="stg", bufs=3))
    ptp = ctx.enter_context(tc.tile_pool(name="pt", bufs=3))
    outp = ctx.enter_context(tc.tile_pool(name="outq", bufs=2))
    rcp = ctx.enter_context(tc.tile_pool(name="rc", bufs=2))
    ps_s = ctx.enter_context(tc.tile_pool(name="ps_s", bufs=2, space="PSUM"))
    ps_m = ctx.enter_context(tc.tile_pool(name="ps_m", bufs=2, space="PSUM"))

    def _body():
        ident = const.tile([P, P], f16)
        make_identity(nc, ident[:])
        # PE warmup: harmless transposes during the initial load DMAs keep the
        # HAM activity window busy so real work starts at full clock.
        wps = ps_m.tile([P, P], f16, name="wps", tag="o", bufs=4)
        for _ in range(24):
            nc.tensor.transpose(wps[:], ident[:], ident[:])

        # ---- mask: fp16 keep-multipliers, pre-transposed host-side, arrive
        # via natural-layout DMA, one [k=128, q=S] tile per k-tile.
        # Emitted AFTER the head-0 loads so they don't delay compute.
        nm = predp.tile([P, NT, S], f16, name="nm")
        m_r = m_d.rearrange("(t p) q -> p t q", p=P)

        def mask_chunk(t, qh):
            # per q-half chunks: only the q0:q0+1024 slice is needed before
            # half-head hh runs, so the front-loaded critical demand halves
            q0 = 1024 * qh
            nc.sync.dma_start(nm[:, t, q0:q0 + 1024],
                              m_r[:, t, q0:q0 + 1024])

        v_r = v_d.rearrange("(j p) h d -> p j h d", p=P)
        o_r = o_d.rearrange("(qd jj p) h d -> qd p jj h d", jj=4, p=P)

        head_res = {}

        def alloc(i):
            QT = stg.tile([P, S], f16, tag="qs")
            KT = stg.tile([P, S], f16, tag="ks")
            VP = stg.tile([P, NT, D + 1], f16, tag="vs")
            head_res[i] = (QT, KT, VP)
            return QT, KT, VP

        def load_qk(i):
            QT, KT, VP = head_res[i]
            nc.sync.dma_start(KT[:], kt_d[i])
            nc.sync.dma_start(QT[:], qt_d[i])

        def load_v(i):
            QT, KT, VP = head_res[i]
            nc.sync.dma_start(VP[:, :, 0:D], v_r[:, :, i, :])
            nc.gpsimd.memset(VP[:, :, D:D + 1], 1.0)

        def mm1_step(i, hh, t, PT):
            QT, KT, VP = head_res[i]
            q0 = (S // 2) * hh
            ps = ps_s.tile([P, 1024], f32)
            nc.tensor.matmul(ps[:, 0:512], KT[:, t * P:(t + 1) * P],
                             QT[:, q0:q0 + 512], start=True, stop=True)
            nc.tensor.matmul(ps[:, 512:1024], KT[:, t * P:(t + 1) * P],
                             QT[:, q0 + 512:q0 + 1024], start=True, stop=True)
            nc.scalar.activation(PT[:, t, :], ps[:], Exp, scale=SCALE)
            if "nomask" not in ablate:
                # per-tile masking multiply right after each act: finer DVE
                # granularity releases PT regions to the mm2 chains sooner
                nc.vector.tensor_tensor(
                    out=PT[:, t, :], in0=PT[:, t, :],
                    in1=nm[:, t, q0:q0 + 1024], op=MUL)

        oq_state = {}

        po_state = {}

        def mm2_half(prev, jj, half, fine_out=False):
            """First/second 8 k-tiles of the context chain for q-tile jj.

            Splitting the 16-step chain in two keeps PE's per-period load
            smooth (one mm1 + one half-chain per step) instead of alternating
            idle/spike periods that head-of-line-block the next half's mm1.
            """
            i, hh, PT = prev
            QT, KT, VP = head_res[i]
            if half == 0:
                po_state[prev[:2]] = ps_m.tile([P, D + 1], f32, name="po",
                                               tag="o", bufs=4)
            po = po_state[prev[:2]]
            t0 = 8 * half
            for t in range(t0, t0 + 8):
                nc.tensor.matmul(po[:], PT[:, t, P * jj:P * (jj + 1)],
                                 VP[:, t, :],
                                 start=(t == 0), stop=(t == NT - 1))
            if half == 0:
                return
            j = 8 * hh + jj  # global q-tile index
            rc = rcp.tile([P, 1], f32)
            nc.vector.reciprocal(rc[:], po[:, D:D + 1])
            quad, sub = divmod(j, 4)
            if sub == 0:
                oq_state[i] = outp.tile([P, 4, D], f32, name="oq", tag="oq")
            oq = oq_state[i]
            nc.vector.tensor_scalar_mul(oq[:, sub, :], po[:, 0:D], rc[:])
            # SP HWDGE, not gpsimd SWDGE: the Pool-engine descriptor
            # generation (~1.2us per transfer) serializes the drain
            if fine_out and sub % 2 == 1:
                # epilogue: ship each q-tile pair as soon as it's scaled, so
                # the drain only waits on a half-size final transfer
                nc.sync.dma_start(o_r[quad, :, sub - 1:sub + 1, i, :],
                                  oq[:, sub - 1:sub + 1, :])
            elif not fine_out and sub == 3:
                nc.sync.dma_start(o_r[quad, :, :, i, :], oq[:])

        def mm2_step(prev, jj):
            mm2_half(prev, jj, 0, fine_out=True)
            mm2_half(prev, jj, 1, fine_out=True)

        # ---- software pipeline over 8 half-heads --------------------------
        # DMA scheduling: all copies funnel through one ~330GB/s pipe, so
        # transfers are emitted in deadline order.  Hard deadlines: q/k of
        # head i before half 2i starts; soft: mask chunk t before its DVE
        # multiply (lateness only delays mm2, which has slack via PT bufs=3).
        halves = [(i, hh) for i in range(HPC) for hh in range(2)]
        QT0, KT0, VP0 = alloc(0)
        alloc(1)
        nc.sync.dma_start(QT0[:, 0:1024], qt_d[0][:, 0:1024])
        nc.sync.dma_start(KT0[:, 0:512], kt_d[0][:, 0:512])
        nc.sync.dma_start(KT0[:, 512:S], kt_d[0][:, 512:S])
        nc.sync.dma_start(QT0[:, 1024:S], qt_d[0][:, 1024:S])
        for t in range(8):
            mask_chunk(t, 0)
        load_qk(1)
        for t in range(8, NT):
            mask_chunk(t, 0)
        load_v(0)
        for t in range(NT):
            mask_chunk(t, 1)
        alloc(2)
        load_qk(2)
        load_v(1)
        alloc(3)
        load_qk(3)
        load_v(2)
        load_v(3)
        prev = None
        for (i, hh) in halves:
            PT = ptp.tile([P, NT, S // 2], f16)
            for x in range(NT):
                mm1_step(i, hh, x, PT)
                if prev is not None:
                    # one half-chain per step: jj = x // 2, halves alternate
                    mm2_half(prev, x // 2, x % 2)
            prev = (i, hh, PT)
        for jj in range(8):
            mm2_step(prev, jj)

    if hw_loop and reps > 1:
        with tc.For_i(0, reps, 1):
            _body()
    else:
        for _rep in range(reps):
            _body()


def build_nc(reps=1, hw_loop=False, ablate=()):
    nc = bacc.Bacc("TRN2", target_bir_lowering=False, debug=False)
    qt_d = nc.dram_tensor("qt", [HPC, D, S], f16, kind="ExternalInput").ap()
    kt_d = nc.dram_tensor("kt", [HPC, D, S], f16, kind="ExternalInput").ap()
    v_d = nc.dram_tensor("v", [S, HPC, D], f16, kind="ExternalInput").ap()
    m_d = nc.dram_tensor("nmask", [S, S], f16, kind="ExternalInput").ap()
    o_d = nc.dram_tensor("out", [S, HPC, D], f32, kind="ExternalOutput").ap()
    with tile.TileContext(nc) as tc, ExitStack() as ctx:
        _emit(ctx, tc, qt_d, kt_d, v_d, m_d, o_d, reps=reps, hw_loop=hw_loop,
              ablate=ablate)
    nc.compile()
    return nc


_nc_cache = None


def get_nc():
    global _nc_cache
    if _nc_cache is None:
        _nc_cache = build_nc()
    return _nc_cache


def make_in_maps(query_layer, key_layer, value_layer, attention_mask):
    q = np.asarray(query_layer, dtype=np.float32)
    k = np.asarray(key_layer, dtype=np.float32)
    v = np.asarray(value_layer, dtype=np.float32)
    m = np.asarray(attention_mask)
    nmask = [np.ascontiguousarray((~m[b, 0]).T.astype(np.float16))
             for b in range(B)]
    in_maps = []
    for c in range(N_CORES):
        b, g = divmod(c, HPC)
        hs = slice(HPC * g, HPC * g + HPC)
        # [S, HPC, D] -> [HPC, D, S] fp16 (per-head transposed)
        qt = np.ascontiguousarray(
            q[:, b, hs, :].astype(np.float16).transpose(1, 2, 0))
        kt = np.ascontiguousarray(
            k[:, b, hs, :].astype(np.float16).transpose(1, 2, 0))
        v16 = np.ascontiguousarray(v[:, b, hs, :].astype(np.float16))
        in_maps.append({
            "qt": qt,
            "kt": kt,
            "v": v16,
            "nmask": nmask[b],
        })
    return in_maps


def assemble(results):
    out = np.empty((S, B, H, D), np.float32)
    for c in range(N_CORES):
        b, g = divmod(c, HPC)
        out[:, b, HPC * g:HPC * g + HPC, :] = results[c]["out"]
    return out.reshape(S, B, H * D)


def kernel(query_layer, key_layer, value_layer, attention_mask):
    nc = get_nc()
    in_maps = make_in_maps(query_layer, key_layer, value_layer, attention_mask)
    res = run_bass_kernel_spmd(nc, in_maps, core_ids=list(range(N_CORES)))
    return assemble(res.results)


# revision 64
# speedup vs baseline: 1.0066x; 1.0066x over previous
"""CoreAttention Trainium2 Bass kernel.

Full inputs -> full output; internally shards (batch, head-group) across 8
NeuronCores: core c handles batch c//4, heads 4*(c%4) .. 4*(c%4)+4.

Per-core algorithm (per head, seq=2048, d=128):
  - scores are computed TRANSPOSED: S^T[k, q] = (K^T).T @ (Q^T) on the PE,
    so that softmax probabilities come out directly in the [k, q] layout that
    the second matmul (context = P @ V) needs as its stationary operand.
  - Q^T and K^T are prepared HOST-SIDE as fp16 [d, s] per head, so the device
    does no transposes and no dtype converts: DMA loads feed the PE directly.
  - softmax skips max-subtraction (logits ~ N(0,1); exp is safe in fp32) and
    the row sums come for free from a ones-column appended to V.  Masked
    entries are zeroed exactly after exp (matching the reference where
    exp(-10000 - max) underflows to 0), and normalization happens on the
    [q, 128] context output via a per-row reciprocal.
  - the boolean mask is converted host-side to a TRANSPOSED fp16
    keep-multiplier [sk, sq] (1.0 = unmasked) and loaded with natural-layout
    DMA into per-k-tile [k, q] multiplier tiles.
  - PE operands are fp16 (1 cycle/row); accumulation is fp32 in PSUM.
"""

from contextlib import ExitStack

import numpy as np

import concourse.bacc as bacc
from concourse import mybir
import concourse.tile as tile
from concourse.bass_utils import run_bass_kernel_spmd
from concourse.masks import make_identity

S, B, H, D = 2048, 2, 16, 128
HPC = 4  # heads per core
N_CORES = 8
P = 128
NT = S // P  # 16 key/query tiles
SCALE = float(1.0 / np.sqrt(D))  # norm_factor = sqrt(d) * layer_number(=1)

f32 = mybir.dt.float32
f16 = mybir.dt.float16

Exp = mybir.ActivationFunctionType.Exp
MUL = mybir.AluOpType.mult


def _emit(ctx, tc, qt_d, kt_d, v_d, m_d, o_d, reps=1, hw_loop=False, ablate=()):
    nc = tc.nc
    const = ctx.enter_context(tc.tile_pool(name="const", bufs=1))
    predp = ctx.enter_context(tc.tile_pool(name="pred", bufs=1))
    stg = ctx.enter_context(tc.tile_pool(name="stg", bufs=3))
    ptp = ctx.enter_context(tc.tile_pool(name="pt", bufs=3))
    outp = ctx.enter_context(tc.tile_pool(name="outq", bufs=2))
    rcp = ctx.enter_context(tc.tile_pool(name="rc", bufs=2))
    ps_s = ctx.enter_context(tc.tile_pool(name="ps_s", bufs=2, space="PSUM"))
    ps_m = ctx.enter_context(tc.tile_pool(name="ps_m", bufs=2, space="PSUM"))

    def _body():
        ident = const.tile([P, P], f16)
        make_identity(nc, ident[:])
        # PE warmup: harmless transposes during the initial load DMAs keep the
        # HAM activity window busy so real work starts at full clock.
        wps = ps_m.tile([P, P], f16, name="wps", tag="o", bufs=4)
        for _ in range(24):
            nc.tensor.transpose(wps[:], ident[:], ident[:])

        # ---- mask: fp16 keep-multipliers, pre-transposed host-side, arrive
        # via natural-layout DMA, one [k=128, q=S] tile per k-tile.
        # Emitted AFTER the head-0 loads so they don't delay compute.
        nm = predp.tile([P, NT, S], f16, name="nm")
        m_r = m_d.rearrange("(t p) q -> p t q", p=P)

        def mask_chunk(t, qh):
            # per q-half chunks: only the q0:q0+1024 slice is needed before
            # half-head hh runs, so the front-loaded critical demand halves
            q0 = 1024 * qh
            nc.sync.dma_start(nm[:, t, q0:q0 + 1024],
                              m_r[:, t, q0:q0 + 1024])

        v_r = v_d.rearrange("(j p) h d -> p j h d", p=P)
        o_r = o_d.rearrange("(qd jj p) h d -> qd p jj h d", jj=4, p=P)

        head_res = {}

        def alloc(i):
            QT = stg.tile([P, S], f16, tag="qs")
            KT = stg.tile([P, S], f16, tag="ks")
            VP = stg.tile([P, NT, D + 1], f16, tag="vs")
            head_res[i] = (QT, KT, VP)
            return QT, KT, VP

        def load_qk(i):
            QT, KT, VP = head_res[i]
            nc.sync.dma_start(KT[:], kt_d[i])
            nc.sync.dma_start(QT[:], qt_d[i])

        def load_v(i):
            QT, KT, VP = head_res[i]
            nc.sync.dma_start(VP[:, :, 0:D], v_r[:, :, i, :])
            nc.gpsimd.memset(VP[:, :, D:D + 1], 1.0)

        def mm1_step(i, hh, t, PT):
            QT, KT, VP = head_res[i]
            q0 = (S // 2) * hh
            ps = ps_s.tile([P, 1024], f32)
            nc.tensor.matmul(ps[:, 0:512], KT[:, t * P:(t + 1) * P],
                             QT[:, q0:q0 + 512], start=True, stop=True)
            nc.tensor.matmul(ps[:, 512:1024], KT[:, t * P:(t + 1) * P],
                             QT[:, q0 + 512:q0 + 1024], start=True, stop=True)
            nc.scalar.activation(PT[:, t, :], ps[:], Exp, scale=SCALE)
            if "nomask" not in ablate:
                # per-tile masking multiply right after each act: finer DVE
                # granularity releases PT regions to the mm2 chains sooner
                nc.vector.tensor_tensor(
                    out=PT[:, t, :], in0=PT[:, t, :],
                    in1=nm[:, t, q0:q0 + 1024], op=MUL)

        oq_state = {}

        po_state = {}

        def mm2_half(prev, jj, half, fine_out=False):
            """First/second 8 k-tiles of the context chain for q-tile jj.

            Splitting the 16-step chain in two keeps PE's per-period load
            smooth (one mm1 + one half-chain per step) instead of alternating
            idle/spike periods that head-of-line-block the next half's mm1.
            """
            i, hh, PT = prev
            QT, KT, VP = head_res[i]
            if half == 0:
                po_state[prev[:2]] = ps_m.tile([P, D + 1], f32, name="po",
                                               tag="o", bufs=4)
            po = po_state[prev[:2]]
            t0 = 8 * half
            for t in range(t0, t0 + 8):
                nc.tensor.matmul(po[:], PT[:, t, P * jj:P * (jj + 1)],
                                 VP[:, t, :],
                                 start=(t == 0), stop=(t == NT - 1))
            if half == 0:
                return
            j = 8 * hh + jj  # global q-tile index
            rc = rcp.tile([P, 1], f32)
            nc.vector.reciprocal(rc[:], po[:, D:D + 1])
            quad, sub = divmod(j, 4)
            if sub == 0:
                oq_state[i] = outp.tile([P, 4, D], f32, name="oq", tag="oq")
            oq = oq_state[i]
            nc.vector.tensor_scalar_mul(oq[:, sub, :], po[:, 0:D], rc[:])
            # SP HWDGE, not gpsimd SWDGE: the Pool-engine descriptor
            # generation (~1.2us per transfer) serializes the drain
            if fine_out and sub % 2 == 1:
                # epilogue: ship each q-tile pair as soon as it's scaled, so
                # the drain only waits on a half-size final transfer
                nc.sync.dma_start(o_r[quad, :, sub - 1:sub + 1, i, :],
                                  oq[:, sub - 1:sub + 1, :])
            elif not fine_out and sub == 3:
                nc.sync.dma_start(o_r[quad, :, :, i, :], oq[:])

        def mm2_step(prev, jj):
            mm2_half(prev, jj, 0, fine_out=True)
            mm2_half(prev, jj, 1, fine_out=True)

        # ---- software pipeline over 8 half-heads --------------------------
        # DMA scheduling: all copies funnel through one ~330GB/s pipe, so
        # transfers are emitted in deadline order.  Hard deadlines: q/k of
        # head i before half 2i starts; soft: mask chunk t before its DVE
        # multiply (lateness only delays mm2, which has slack via PT bufs=3).
        halves = [(i, hh) for i in range(HPC) for hh in range(2)]
        QT0, KT0, VP0 = alloc(0)
        alloc(1)
        nc.sync.dma_start(QT0[:, 0:1024], qt_d[0][:, 0:1024])
        nc.sync.dma_start(KT0[:, 0:512], kt_d[0][:, 0:512])
        nc.sync.dma_start(KT0[:, 512:S], kt_d[0][:, 512:S])
        nc.sync.dma_start(QT0[:, 1024:S], qt_d[0][:, 1024:S])
        for t in range(8):
            mask_chunk(t, 0)
        load_qk(1)
        for t in range(8, NT):
            mask_chunk(t, 0)
        load_v(0)
        for t in range(NT):
            mask_chunk(t, 1)
        alloc(2)
        load_qk(2)
        load_v(1)
        alloc(3)
        load_qk(3)
        load_v(2)
        load_v(3)
        prev = None
        for (i, hh) in halves:
            PT = ptp.tile([P, NT, S // 2], f16)
            for x in range(NT):
                mm1_step(i, hh, x, PT)
                if prev is not None:
                    # one half-chain per step: jj = x // 2, halves alternate
                    mm2_half(prev, x // 2, x % 2)
            prev = (i, hh, PT)
        for jj in range(8):
            mm2_step(prev, jj)

    if hw_loop and reps > 1:
        with tc.For_i(0, reps, 1):
            _body()
    else:
        for _rep in range(reps):
            _body()


def build_nc(reps=1, hw_loop=False, ablate=()):
    nc = bacc.Bacc("TRN2", target_bir_lowering=False, debug=False)
    qt_d = nc.dram_tensor("qt", [HPC, D, S], f16, kind="ExternalInput").ap()
    kt_d = nc.dram_tensor("kt", [HPC, D, S], f16, kind="ExternalInput").ap()
    v_d = nc.dram_tensor("v", [S, HPC, D], f16, kind="ExternalInput").ap()
    m_d = nc.dram_tensor("nmask", [S, S], f16, kind="ExternalInput").ap()
    o_d = nc.dram_tensor("out", [S, HPC, D], f32, kind="ExternalOutput").ap()
    with tile.TileContext(nc) as tc, ExitStack() as ctx:
        _emit(ctx, tc, qt_d, kt_d, v_d, m_d, o_d, reps=reps, hw_loop=hw_loop,
              ablate=ablate)
    nc.compile()
    return nc


_nc_cache = None


def get_nc():
    global _nc_cache
    if _nc_cache is None:
        _nc_cache = build_nc()
    return _nc_cache


def make_in_maps(query_layer, key_layer, value_layer, attention_mask):
    q = np.asarray(query_layer, dtype=np.float32)
    k = np.asarray(key_layer, dtype=np.float32)
    v = np.asarray(value_layer, dtype=np.float32)
    m = np.asarray(attention_mask)
    nmask = [np.ascontiguousarray((~m[b, 0]).T.astype(np.float16))
             for b in range(B)]
    in_maps = []
    for c in range(N_CORES):
        b, g = divmod(c, HPC)
        hs = slice(HPC * g, HPC * g + HPC)
        # [S, HPC, D] -> [HPC, D, S] fp16 (per-head transposed)
        qt = np.ascontiguousarray(
            q[:, b, hs, :].astype(np.float16).transpose(1, 2, 0))
        kt = np.ascontiguousarray(
            k[:, b, hs, :].astype(np.float16).transpose(1, 2, 0))
        v16 = np.ascontiguousarray(v[:, b, hs, :].astype(np.float16))
        in_maps.append({
            "qt": qt,
            "kt": kt,
            "v": v16,
            "nmask": nmask[b],
        })
    return in_maps


def assemble(results):
    out = np.empty((S, B, H, D), np.float32)
    for c in range(N_CORES):
        b, g = divmod(c, HPC)
        out[:, b, HPC * g:HPC * g + HPC, :] = results[c]["out"]
    return out.reshape(S, B, H * D)


def kernel(query_layer, key_layer, value_layer, attention_mask):
    nc = get_nc()
    in_maps = make_in_maps(query_layer, key_layer, value_layer, attention_mask)
    res = run_bass_kernel_spmd(nc, in_maps, core_ids=list(range(N_CORES)))
    return assemble(res.results)
